# revision 20
# baseline (speedup 1.0000x reference)
"""DMPNet Trainium2 kernel.

Strategy
--------
* Pure batch data parallelism: 16384 rows -> 8 cores x 2048.
* The MLP (128 -> 2048 -> 2048 -> 54, tanh) runs on the tensor engine:
  layer-1/final weights and h0/h1 activations in bf16 (halves SBUF +
  weight DMA; ~3.5e-3 rel err, gate is 2e-2), layer-0 and the tail
  combine in float32r.  The PE cannot mix 32-bit and 16-bit matmul
  operands, so each matmul is a matched-dtype pair; all run 1 elem/cycle.
* The 101-step DMP Euler integration is a linear time-invariant recurrence
  in (y, z); it collapses exactly into
      out[r, j] = da_j*y0 + db_j*dy0 + dg_j*goal + (goal - y0) * (w @ dQ_j)
  with coefficients precomputed on the host in float64.  The (w @ dQ) part
  is folded into the final-layer weights (W_eff), so the device only runs
  3 matmul layers + 2 tiny broadcast matmuls + 2 elementwise ops.
* All activations live feature-major ([feature, batch]); the input is
  transposed host-side and staged ONCE into SBUF (x_all) - the steady-state
  pass has zero input DMAs.  Layer-0 chunks for the next batch tile are
  spread one-per-j through the layer-1 j-loop so the scalar engine's tanh
  throughput never gates the PE.  h0 lives in two persistent 16-tile
  generations that ping-pong across batch tiles (and across For_i
  iterations for the timing loop).
"""

import os

import ml_dtypes
import numpy as np

import concourse.bass as bass
import concourse.mybir as mybir
from concourse import bacc
from concourse.tile import TileContext
from concourse.bass_utils import run_bass_kernel_spmd

F32 = mybir.dt.float32
F32R = mybir.dt.float32r
BF16 = mybir.dt.bfloat16

N_CORES = 8
B_TOTAL = 16384
B_SH = B_TOTAL // N_CORES          # 2048 rows per core
D_IN = 128
H = 2048
HC = H // 128                      # 16 chunks of 128
DIM = 9
N_BASIS = 5
NOUT = 10                          # output time steps
M_S = DIM * NOUT                   # 90 "S" rows
M_ALL = M_S + DIM                  # 99 rows of the effective final layer

TW = int(os.environ.get("DMP_TW", "512"))            # batch tile width
REPEAT = int(os.environ.get("DMP_KERNEL_REPEAT", "1"))
FORI_REPS = int(os.environ.get("DMP_FORI_REPS", "1"))  # hardware-loop reps (timing)
STAGGER = int(os.environ.get("DMP_STAGGER", "1"))     # staggered-reset For_i
NT = B_SH // TW

_TANH = mybir.ActivationFunctionType.Tanh


def _round_fp32r(x: np.ndarray) -> np.ndarray:
    """Round fp32 -> fp32r (11 explicit mantissa bits), nearest-even."""
    b = np.ascontiguousarray(x, dtype=np.float32).view(np.uint32)
    lsb = (b >> np.uint32(12)) & np.uint32(1)
    r = b + (np.uint32(0x7FF) + lsb)
    r &= np.uint32(0xFFFFF000)
    return r.view(np.float32)


def _bf16(x: np.ndarray) -> np.ndarray:
    return np.ascontiguousarray(np.asarray(x, np.float32)).astype(ml_dtypes.bfloat16)


def _dmp_coefficients():
    """Closed-form coefficients of the sampled-position differences.

    Returns (d_alpha, d_beta, d_gamma, dQ) with dQ shaped (NOUT, N_BASIS):
      out[r, j] = d_alpha[j]*y0 + d_beta[j]*dy0 + d_gamma[j]*goal
                  + (goal - y0) * sum_n w[r, n] * dQ[j, n]
    """
    A_X, A_Z, TAU, DT = 1.0, 25.0, 1.0, 0.01
    B_Z = A_Z / 4.0
    NSTEP, L_SUB = 100, 10

    c = np.exp(-A_X * np.linspace(0.0, 1.0, N_BASIS))
    h = N_BASIS ** 1.5 / c / A_X
    xs = (1.0 - A_X * DT / TAU) ** np.arange(1, NSTEP + 1)
    psi = np.exp(-h[None, :] * (xs[:, None] - c[None, :]) ** 2)
    p = psi * xs[:, None] / psi.sum(axis=1, keepdims=True)      # (100, 5)

    nb = 3 + NSTEP
    cy = np.zeros(nb)
    cz = np.zeros(nb)
    cy[0] = 1.0
    cz[1] = TAU
    ys = [cy.copy()]
    for k in range(NSTEP):
        dz = np.zeros(nb)
        dz[2] = A_Z * B_Z
        dz -= A_Z * B_Z * cy
        dz -= A_Z * cz
        dz[3 + k] += 1.0
        dz /= TAU
        dy = cz / TAU
        cy = cy + dy * DT
        cz = cz + dz * DT
        ys.append(cy.copy())
    ys = np.array(ys)                         # (101, 103)
    samp = ys[::L_SUB]                        # (11, 103)
    dcoef = samp[1:] - samp[:-1]              # (10, 103)
    dQ = dcoef[:, 3:] @ p                     # (10, 5)
    return dcoef[:, 0], dcoef[:, 1], dcoef[:, 2], dQ


_NC_CACHE = {}


def _build_program(tw: int, repeat: int, fori_reps: int = 1, stagger: int = STAGGER):
    nt = B_SH // tw
    # one h0 generation per batch tile, written two tiles ahead; the
    # staggered-reset stage-adjacency invariant (stage I waits on I-2)
    # makes the cross-iteration handoff race-free only at distance 2
    assert nt == 4, f"nt={nt} must be 4"
    nc = bacc.Bacc()

    xT = nc.dram_tensor("xT", [D_IN, B_SH], F32R, kind="ExternalInput")
    w0t = nc.dram_tensor("w0t", [D_IN, H], F32R, kind="ExternalInput")
    b0d = nc.dram_tensor("b0d", [128, HC], F32, kind="ExternalInput")
    w1t = nc.dram_tensor("w1t", [H, H], BF16, kind="ExternalInput")
    b1d = nc.dram_tensor("b1d", [128, HC], F32, kind="ExternalInput")
    weff = nc.dram_tensor("weff", [H, M_ALL], BF16, kind="ExternalInput")
    beff = nc.dram_tensor("beff", [M_ALL, 1], F32, kind="ExternalInput")
    linc = nc.dram_tensor("linc", [117, M_S], F32R, kind="ExternalInput")
    diffc = nc.dram_tensor("diffc", [117, M_S], F32R, kind="ExternalInput")
    outT = nc.dram_tensor("outT", [M_S, B_SH], F32, kind="ExternalOutput")

    with TileContext(nc) as tc:
        with (
            tc.tile_pool(name="wres", bufs=1) as wres,
            tc.tile_pool(name="h1p", bufs=3) as h1p,
            tc.tile_pool(name="outp", bufs=4) as outp,
            tc.tile_pool(name="ps_l0", bufs=2, space="PSUM") as ps_l0,
            tc.tile_pool(name="ps_h1", bufs=4, space="PSUM") as ps_h1,
            tc.tile_pool(name="ps_m", bufs=2, space="PSUM") as ps_m,
        ):
            # ---- resident weights / constants / input ----
            w0_sb = wres.tile([128, H], F32R, tag="w0")
            nc.sync.dma_start(out=w0_sb, in_=w0t[:, :])
            b0_sb = wres.tile([128, HC], F32, tag="b0")
            nc.sync.dma_start(out=b0_sb, in_=b0d[:, :])
            x_all = wres.tile([128, B_SH], F32R, tag="xall")
            nc.sync.dma_start(out=x_all, in_=xT[:, :])
            b1_sb = wres.tile([128, HC], F32, tag="b1")
            nc.sync.dma_start(out=b1_sb, in_=b1d[:, :])
            # j-major 128x128 blocks: the layer-1 j-loop consumes w1[:, :, j*128]
            # column blocks in order, so tile 0's j-loop can start as soon as
            # the first blocks land instead of waiting for the whole 8 MB
            w1_sb = wres.tile([128, HC, H], BF16, tag="w1")
            for j in range(HC):
                for i in range(HC):
                    nc.sync.dma_start(
                        out=w1_sb[:, i, j * 128:(j + 1) * 128],
                        in_=w1t[i * 128:(i + 1) * 128, j * 128:(j + 1) * 128],
                    )
            weff_sb = wres.tile([128, HC, M_ALL], BF16, tag="weff")
            for i in range(HC):
                nc.sync.dma_start(out=weff_sb[:, i, :], in_=weff[i * 128:(i + 1) * 128, :])
            beff_sb = wres.tile([M_ALL, 1], F32, tag="beff")
            nc.sync.dma_start(out=beff_sb, in_=beff[:, :])
            linc_sb = wres.tile([117, M_S], F32R, tag="linc")
            nc.sync.dma_start(out=linc_sb, in_=linc[:, :])
            diffc_sb = wres.tile([117, M_S], F32R, tag="diffc")
            nc.sync.dma_start(out=diffc_sb, in_=diffc[:, :])

            # persistent h0 double generation + per-tile m2 combine tiles
            h0g = [
                [
                    wres.tile(
                        [128, tw], BF16, tag=f"h0_{g}_{c}", name=f"h0_{g}_{c}"
                    )
                    for c in range(HC)
                ]
                for g in range(nt)
            ]
            m2t = [
                wres.tile([117, tw], F32R, tag=f"m2_{t}", name=f"m2_{t}")
                for t in range(nt)
            ]
            for t in range(nt):
                win = slice(t * tw, (t + 1) * tw)
                nc.sync.dma_start(out=m2t[t][99:108, :], in_=xT[7:16, win])
                nc.sync.dma_start(out=m2t[t][108:117, :], in_=xT[22:31, win])

            def l0_chunk(dst, t_target, c):
                """h0 chunk c for batch tile t_target -> persistent tile dst."""
                win = slice(t_target * tw, (t_target + 1) * tw)
                ps = ps_l0.tile([128, tw], F32, tag="l0")
                nc.tensor.matmul(
                    ps, w0_sb[:, c * 128:(c + 1) * 128], x_all[:, win],
                    start=True, stop=True,
                )
                nc.scalar.activation(
                    out=dst, in_=ps, func=_TANH, bias=b0_sb[:, c:c + 1],
                )

            # preamble: generations 0/1 = tiles 0/1's h0
            for c in range(HC):
                l0_chunk(h0g[0][c], 0, c)
            for c in range(HC):
                l0_chunk(h0g[1][c], 1, c)

            def tail(t):
                win = slice(t * tw, (t + 1) * tw)
                m2 = m2t[t]
                lin_ps = ps_l0.tile([M_S, tw], F32, tag="l0")
                nc.tensor.matmul(lin_ps, linc_sb, m2[0:117, :], start=True, stop=True)
                diff_ps = ps_l0.tile([M_S, tw], F32, tag="l0")
                nc.tensor.matmul(diff_ps, diffc_sb, m2[0:117, :], start=True, stop=True)
                prod = outp.tile([M_S, tw], F32, tag="prod")
                nc.vector.tensor_mul(prod, diff_ps, m2[0:M_S, :].bitcast(F32))
                res = outp.tile([M_S, tw], F32, tag="res")
                nc.vector.tensor_add(res, prod, lin_ps)
                nc.sync.dma_start(out=outT[:, win], in_=res)

            def _one_pass(staged=False):
                pend = None
                for t in range(nt):
                    if staged and t > 0:
                        tc.stage_boundary()
                    gen = h0g[t]
                    ngen = h0g[(t + 2) % nt]
                    nxt = (t + 2) % nt
                    psm = ps_m.tile([M_ALL, tw], F32, tag="m")
                    for j in range(HC):
                        ps1 = ps_h1.tile([128, tw], F32, tag="h1")
                        for i in range(HC):
                            nc.tensor.matmul(
                                ps1, w1_sb[:, i, j * 128:(j + 1) * 128], gen[i],
                                start=(i == 0), stop=(i == HC - 1),
                            )
                        h1c = h1p.tile([128, tw], BF16, tag="h1c")
                        nc.scalar.activation(
                            out=h1c, in_=ps1, func=_TANH, bias=b1_sb[:, j:j + 1],
                        )
                        nc.tensor.matmul(
                            psm, weff_sb[:, j, :], h1c,
                            start=(j == 0), stop=(j == HC - 1),
                            skip_group_check=True,
                        )
                        # layer-0 for tile t+2 (wraps into the next pass for
                        # t >= 2; x is identical every pass), one chunk per
                        # j so ACT tanh never gates the PE
                        l0_chunk(ngen[j], nxt, j)
                        if j == 0 and pend is not None:
                            tail(pend)
                            pend = None
                    # bias-add on the vector engine (ACT queue stays short)
                    nc.vector.tensor_scalar_add(
                        out=m2t[t][0:M_ALL, :],
                        in0=psm,
                        scalar1=beff_sb[:, 0:1],
                    )
                    pend = t
                tail(pend)

            if fori_reps > 1:
                with tc.For_i(
                    0, fori_reps, 1,
                    hint_engines=(mybir.EngineType.PE,),
                    staggered_reset=bool(stagger),
                ):
                    _one_pass(staged=bool(stagger))
            else:
                for _rep in range(repeat):
                    _one_pass()

    nc.compile()
    return nc


def _get_program(tw: int = TW, repeat: int = REPEAT, fori_reps: int = FORI_REPS):
    key = (tw, repeat, fori_reps, STAGGER)
    if key not in _NC_CACHE:
        _NC_CACHE[key] = _build_program(tw, repeat, fori_reps, STAGGER)
    return _NC_CACHE[key]


def _prepare_host_inputs(input, W0, b0, W1, b1, Wl, bl):
    """Build the per-core input maps (host-side prep, float64 coefficients)."""
    input, W0, b0, W1, b1, Wl, bl = (
        np.asarray(a) for a in (input, W0, b0, W1, b1, Wl, bl)
    )
    d_alpha, d_beta, d_gamma, dQ = _dmp_coefficients()

    Wl100 = Wl.astype(np.float64) * 100.0          # (54, H)
    bl100 = bl.astype(np.float64) * 100.0          # (54,)

    # effective final layer: rows 0..89 = S rows (d*10+j), 90..98 = goal rows
    weff = np.zeros((H, M_ALL), dtype=np.float64)
    beff = np.zeros((M_ALL,), dtype=np.float64)
    for d in range(DIM):
        for j in range(NOUT):
            m = d * NOUT + j
            wrow = np.zeros(H, dtype=np.float64)
            brow = 0.0
            for n in range(N_BASIS):
                wrow += dQ[j, n] * Wl100[DIM + N_BASIS * d + n]
                brow += dQ[j, n] * bl100[DIM + N_BASIS * d + n]
            weff[:, m] = wrow
            beff[m] = brow
        weff[:, M_S + d] = Wl100[d]
        beff[M_S + d] = bl100[d]

    # broadcast matmul constants [117, 90]: rhs is the combined mlp2 tile
    # (rows 0..89 = S [zero coeff], 90..98 = goal, 99..107 = y0, 108..116 = dy0)
    linc = np.zeros((117, M_S), dtype=np.float64)
    diffc = np.zeros((117, M_S), dtype=np.float64)
    for d in range(DIM):
        for j in range(NOUT):
            m = d * NOUT + j
            linc[90 + d, m] = d_gamma[j]
            linc[99 + d, m] = d_alpha[j]
            linc[108 + d, m] = d_beta[j]
            diffc[90 + d, m] = 1.0
            diffc[99 + d, m] = -1.0

    shared = {
        "w0t": _round_fp32r(np.ascontiguousarray(W0.T)),
        "b0d": np.ascontiguousarray(np.asarray(b0, np.float32).reshape(HC, 128).T),
        "w1t": _bf16(W1.T),
        "b1d": np.ascontiguousarray(np.asarray(b1, np.float32).reshape(HC, 128).T),
        "weff": _bf16(weff),
        "beff": np.ascontiguousarray(beff.astype(np.float32).reshape(M_ALL, 1)),
        "linc": _round_fp32r(linc),
        "diffc": _round_fp32r(diffc),
    }

    xr = _round_fp32r(np.asarray(input, np.float32))
    in_maps = []
    for c in range(N_CORES):
        m = dict(shared)
        m["xT"] = np.ascontiguousarray(xr[c * B_SH:(c + 1) * B_SH, :].T)
        in_maps.append(m)
    return in_maps


def kernel(input, W0, b0, W1, b1, Wl, bl):
    nc = _get_program()
    in_maps = _prepare_host_inputs(input, W0, b0, W1, b1, Wl, bl)
    results = run_bass_kernel_spmd(nc, in_maps, core_ids=list(range(N_CORES)))
    outs = []
    for c in range(N_CORES):
        o = results.results[c]["outT"]                     # (90, 2048)
        outs.append(o.reshape(DIM, NOUT, B_SH).transpose(2, 0, 1))
    return np.ascontiguousarray(np.concatenate(outs, axis=0), dtype=np.float32)


# revision 27
# speedup vs baseline: 1.0122x; 1.0122x over previous
"""DMPNet Trainium2 kernel.

Strategy
--------
* Pure batch data parallelism: 16384 rows -> 8 cores x 2048.
* The MLP (128 -> 2048 -> 2048 -> 54, tanh) runs on the tensor engine
  entirely in bf16 operands with fp32 PSUM accumulation (~4.8e-3 rel
  err, gate is 2e-2).  bf16 halves SBUF + weight DMA and gets the
  fast-weight-load path on every matmul; all matmuls stream 1 col/cycle.
* The 101-step DMP Euler integration is a linear time-invariant recurrence
  in (y, z); it collapses exactly into
      out[r, j] = da_j*y0 + db_j*dy0 + dg_j*goal + (goal - y0) * (w @ dQ_j)
  with coefficients precomputed on the host in float64.  The (w @ dQ) part
  is folded into the final-layer weights (W_eff), so the device only runs
  3 matmul layers + 2 tiny broadcast matmuls + 2 elementwise ops.
* All activations live feature-major ([feature, batch]); the input is
  transposed host-side and staged ONCE into SBUF (x_all) - the steady-state
  pass has zero input DMAs.  Layer-0 chunks for the next batch tile are
  spread one-per-j through the layer-1 j-loop so the scalar engine's tanh
  throughput never gates the PE.  h0 lives in two persistent 16-tile
  generations that ping-pong across batch tiles (and across For_i
  iterations for the timing loop).
"""

import os

import ml_dtypes
import numpy as np

import concourse.bass as bass
import concourse.mybir as mybir
from concourse import bacc
from concourse.tile import TileContext
from concourse.bass_utils import run_bass_kernel_spmd

F32 = mybir.dt.float32
F32R = mybir.dt.float32r
BF16 = mybir.dt.bfloat16

N_CORES = 8
B_TOTAL = 16384
B_SH = B_TOTAL // N_CORES          # 2048 rows per core
D_IN = 128
H = 2048
HC = H // 128                      # 16 chunks of 128
DIM = 9
N_BASIS = 5
NOUT = 10                          # output time steps
M_S = DIM * NOUT                   # 90 "S" rows
M_ALL = M_S + DIM                  # 99 rows of the effective final layer

TW = int(os.environ.get("DMP_TW", "512"))            # batch tile width
REPEAT = int(os.environ.get("DMP_KERNEL_REPEAT", "1"))
FORI_REPS = int(os.environ.get("DMP_FORI_REPS", "1"))  # hardware-loop reps (timing)
STAGGER = int(os.environ.get("DMP_STAGGER", "1"))     # staggered-reset For_i
NT = B_SH // TW

_TANH = mybir.ActivationFunctionType.Tanh


def _round_fp32r(x: np.ndarray) -> np.ndarray:
    """Round fp32 -> fp32r (11 explicit mantissa bits), nearest-even."""
    b = np.ascontiguousarray(x, dtype=np.float32).view(np.uint32)
    lsb = (b >> np.uint32(12)) & np.uint32(1)
    r = b + (np.uint32(0x7FF) + lsb)
    r &= np.uint32(0xFFFFF000)
    return r.view(np.float32)


def _bf16(x: np.ndarray) -> np.ndarray:
    return np.ascontiguousarray(np.asarray(x, np.float32)).astype(ml_dtypes.bfloat16)


def _dmp_coefficients():
    """Closed-form coefficients of the sampled-position differences.

    Returns (d_alpha, d_beta, d_gamma, dQ) with dQ shaped (NOUT, N_BASIS):
      out[r, j] = d_alpha[j]*y0 + d_beta[j]*dy0 + d_gamma[j]*goal
                  + (goal - y0) * sum_n w[r, n] * dQ[j, n]
    """
    A_X, A_Z, TAU, DT = 1.0, 25.0, 1.0, 0.01
    B_Z = A_Z / 4.0
    NSTEP, L_SUB = 100, 10

    c = np.exp(-A_X * np.linspace(0.0, 1.0, N_BASIS))
    h = N_BASIS ** 1.5 / c / A_X
    xs = (1.0 - A_X * DT / TAU) ** np.arange(1, NSTEP + 1)
    psi = np.exp(-h[None, :] * (xs[:, None] - c[None, :]) ** 2)
    p = psi * xs[:, None] / psi.sum(axis=1, keepdims=True)      # (100, 5)

    nb = 3 + NSTEP
    cy = np.zeros(nb)
    cz = np.zeros(nb)
    cy[0] = 1.0
    cz[1] = TAU
    ys = [cy.copy()]
    for k in range(NSTEP):
        dz = np.zeros(nb)
        dz[2] = A_Z * B_Z
        dz -= A_Z * B_Z * cy
        dz -= A_Z * cz
        dz[3 + k] += 1.0
        dz /= TAU
        dy = cz / TAU
        cy = cy + dy * DT
        cz = cz + dz * DT
        ys.append(cy.copy())
    ys = np.array(ys)                         # (101, 103)
    samp = ys[::L_SUB]                        # (11, 103)
    dcoef = samp[1:] - samp[:-1]              # (10, 103)
    dQ = dcoef[:, 3:] @ p                     # (10, 5)
    return dcoef[:, 0], dcoef[:, 1], dcoef[:, 2], dQ


_NC_CACHE = {}


def _build_program(tw: int, repeat: int, fori_reps: int = 1, stagger: int = STAGGER):
    nt = B_SH // tw
    # one h0 generation per batch tile, written two tiles ahead; the
    # staggered-reset stage-adjacency invariant (stage I waits on I-2)
    # makes the cross-iteration handoff race-free only at distance 2
    assert nt == 4, f"nt={nt} must be 4"
    nc = bacc.Bacc()

    xb = nc.dram_tensor("xb", [D_IN, B_SH], BF16, kind="ExternalInput")
    ydy = nc.dram_tensor("ydy", [18, B_SH], BF16, kind="ExternalInput")
    w0t = nc.dram_tensor("w0t", [D_IN, H], BF16, kind="ExternalInput")
    b0d = nc.dram_tensor("b0d", [128, HC], F32, kind="ExternalInput")
    w1t = nc.dram_tensor("w1t", [H, H], BF16, kind="ExternalInput")
    b1d = nc.dram_tensor("b1d", [128, HC], F32, kind="ExternalInput")
    weff = nc.dram_tensor("weff", [H, M_ALL], BF16, kind="ExternalInput")
    beff = nc.dram_tensor("beff", [M_ALL, 1], F32, kind="ExternalInput")
    linc = nc.dram_tensor("linc", [117, M_S], BF16, kind="ExternalInput")
    diffc = nc.dram_tensor("diffc", [117, M_S], BF16, kind="ExternalInput")
    outT = nc.dram_tensor("outT", [M_S, B_SH], F32, kind="ExternalOutput")

    with TileContext(nc) as tc:
        with (
            tc.tile_pool(name="wres", bufs=1) as wres,
            tc.tile_pool(name="h1p", bufs=3) as h1p,
            tc.tile_pool(name="outp", bufs=4) as outp,
            tc.tile_pool(name="ps_l0", bufs=2, space="PSUM") as ps_l0,
            tc.tile_pool(name="ps_h1", bufs=4, space="PSUM") as ps_h1,
            tc.tile_pool(name="ps_m", bufs=2, space="PSUM") as ps_m,
        ):
            # ---- resident weights / constants / input ----
            w0_sb = wres.tile([128, H], BF16, tag="w0")
            nc.sync.dma_start(out=w0_sb, in_=w0t[:, :])
            b0_sb = wres.tile([128, HC], F32, tag="b0")
            nc.sync.dma_start(out=b0_sb, in_=b0d[:, :])
            x_all = wres.tile([128, B_SH], BF16, tag="xall")
            nc.sync.dma_start(out=x_all, in_=xb[:, :])
            b1_sb = wres.tile([128, HC], F32, tag="b1")
            nc.sync.dma_start(out=b1_sb, in_=b1d[:, :])
            # j-major 128x128 blocks: the layer-1 j-loop consumes w1[:, :, j*128]
            # column blocks in order, so tile 0's j-loop can start as soon as
            # the first blocks land instead of waiting for the whole 8 MB
            w1_sb = wres.tile([128, HC, H], BF16, tag="w1")
            for j in range(HC):
                for i in range(HC):
                    nc.sync.dma_start(
                        out=w1_sb[:, i, j * 128:(j + 1) * 128],
                        in_=w1t[i * 128:(i + 1) * 128, j * 128:(j + 1) * 128],
                    )
            weff_sb = wres.tile([128, HC, M_ALL], BF16, tag="weff")
            for i in range(HC):
                nc.sync.dma_start(out=weff_sb[:, i, :], in_=weff[i * 128:(i + 1) * 128, :])
            beff_sb = wres.tile([M_ALL, 1], F32, tag="beff")
            nc.sync.dma_start(out=beff_sb, in_=beff[:, :])
            linc_sb = wres.tile([117, M_S], BF16, tag="linc")
            nc.sync.dma_start(out=linc_sb, in_=linc[:, :])
            diffc_sb = wres.tile([117, M_S], BF16, tag="diffc")
            nc.sync.dma_start(out=diffc_sb, in_=diffc[:, :])

            # persistent h0 double generation + per-tile m2 combine tiles
            h0g = [
                [
                    wres.tile(
                        [128, tw], BF16, tag=f"h0_{g}_{c}", name=f"h0_{g}_{c}"
                    )
                    for c in range(HC)
                ]
                for g in range(nt)
            ]
            m2t = [
                wres.tile([117, tw], BF16, tag=f"m2_{t}", name=f"m2_{t}")
                for t in range(nt)
            ]
            for t in range(nt):
                win = slice(t * tw, (t + 1) * tw)
                nc.sync.dma_start(out=m2t[t][99:117, :], in_=ydy[:, win])

            def l0_chunk(dst, t_target, c):
                """h0 chunk c for batch tile t_target -> persistent tile dst."""
                win = slice(t_target * tw, (t_target + 1) * tw)
                ps = ps_l0.tile([128, tw], F32, tag="l0")
                nc.tensor.matmul(
                    ps, w0_sb[:, c * 128:(c + 1) * 128], x_all[:, win],
                    start=True, stop=True,
                )
                nc.scalar.activation(
                    out=dst, in_=ps, func=_TANH, bias=b0_sb[:, c:c + 1],
                )

            # preamble: generations 0/1 = tiles 0/1's h0
            for c in range(HC):
                l0_chunk(h0g[0][c], 0, c)
            for c in range(HC):
                l0_chunk(h0g[1][c], 1, c)

            def tail(t):
                win = slice(t * tw, (t + 1) * tw)
                m2 = m2t[t]
                lin_ps = ps_l0.tile([M_S, tw], F32, tag="l0")
                nc.tensor.matmul(lin_ps, linc_sb, m2[0:117, :], start=True, stop=True)
                diff_ps = ps_l0.tile([M_S, tw], F32, tag="l0")
                nc.tensor.matmul(diff_ps, diffc_sb, m2[0:117, :], start=True, stop=True)
                prod = outp.tile([M_S, tw], F32, tag="prod")
                nc.vector.tensor_mul(prod, diff_ps, m2[0:M_S, :])
                res = outp.tile([M_S, tw], F32, tag="res")
                nc.vector.tensor_add(res, prod, lin_ps)
                nc.sync.dma_start(out=outT[:, win], in_=res)

            def _one_pass(staged=False):
                pend = None
                for t in range(nt):
                    if staged and t > 0:
                        tc.stage_boundary()
                    gen = h0g[t]
                    ngen = h0g[(t + 2) % nt]
                    nxt = (t + 2) % nt
                    psm = ps_m.tile([M_ALL, tw], F32, tag="m")
                    for j in range(HC):
                        ps1 = ps_h1.tile([128, tw], F32, tag="h1")
                        for i in range(HC):
                            nc.tensor.matmul(
                                ps1, w1_sb[:, i, j * 128:(j + 1) * 128], gen[i],
                                start=(i == 0), stop=(i == HC - 1),
                            )
                        h1c = h1p.tile([128, tw], BF16, tag="h1c")
                        nc.scalar.activation(
                            out=h1c, in_=ps1, func=_TANH, bias=b1_sb[:, j:j + 1],
                        )
                        nc.tensor.matmul(
                            psm, weff_sb[:, j, :], h1c,
                            start=(j == 0), stop=(j == HC - 1),
                            skip_group_check=True,
                        )
                        # layer-0 for tile t+2 (wraps into the next pass for
                        # t >= 2; x is identical every pass), one chunk per
                        # j so ACT tanh never gates the PE
                        l0_chunk(ngen[j], nxt, j)
                        if j == 0 and pend is not None:
                            tail(pend)
                            pend = None
                    # bias-add on the vector engine (ACT queue stays short)
                    nc.vector.tensor_scalar_add(
                        out=m2t[t][0:M_ALL, :],
                        in0=psm,
                        scalar1=beff_sb[:, 0:1],
                    )
                    pend = t
                tail(pend)

            if fori_reps > 1:
                with tc.For_i(
                    0, fori_reps, 1,
                    hint_engines=(mybir.EngineType.PE,),
                    staggered_reset=bool(stagger),
                ):
                    _one_pass(staged=bool(stagger))
            else:
                for _rep in range(repeat):
                    _one_pass()

    nc.compile()
    return nc


def _get_program(tw: int = TW, repeat: int = REPEAT, fori_reps: int = FORI_REPS):
    key = (tw, repeat, fori_reps, STAGGER)
    if key not in _NC_CACHE:
        _NC_CACHE[key] = _build_program(tw, repeat, fori_reps, STAGGER)
    return _NC_CACHE[key]


def _prepare_host_inputs(input, W0, b0, W1, b1, Wl, bl):
    """Build the per-core input maps (host-side prep, float64 coefficients)."""
    input, W0, b0, W1, b1, Wl, bl = (
        np.asarray(a) for a in (input, W0, b0, W1, b1, Wl, bl)
    )
    d_alpha, d_beta, d_gamma, dQ = _dmp_coefficients()

    Wl100 = Wl.astype(np.float64) * 100.0          # (54, H)
    bl100 = bl.astype(np.float64) * 100.0          # (54,)

    # effective final layer: rows 0..89 = S rows (d*10+j), 90..98 = goal rows
    weff = np.zeros((H, M_ALL), dtype=np.float64)
    beff = np.zeros((M_ALL,), dtype=np.float64)
    for d in range(DIM):
        for j in range(NOUT):
            m = d * NOUT + j
            wrow = np.zeros(H, dtype=np.float64)
            brow = 0.0
            for n in range(N_BASIS):
                wrow += dQ[j, n] * Wl100[DIM + N_BASIS * d + n]
                brow += dQ[j, n] * bl100[DIM + N_BASIS * d + n]
            weff[:, m] = wrow
            beff[m] = brow
        weff[:, M_S + d] = Wl100[d]
        beff[M_S + d] = bl100[d]

    # broadcast matmul constants [117, 90]: rhs is the combined mlp2 tile
    # (rows 0..89 = S [zero coeff], 90..98 = goal, 99..107 = y0, 108..116 = dy0)
    linc = np.zeros((117, M_S), dtype=np.float64)
    diffc = np.zeros((117, M_S), dtype=np.float64)
    for d in range(DIM):
        for j in range(NOUT):
            m = d * NOUT + j
            linc[90 + d, m] = d_gamma[j]
            linc[99 + d, m] = d_alpha[j]
            linc[108 + d, m] = d_beta[j]
            diffc[90 + d, m] = 1.0
            diffc[99 + d, m] = -1.0

    shared = {
        "w0t": _bf16(W0.T),
        "b0d": np.ascontiguousarray(np.asarray(b0, np.float32).reshape(HC, 128).T),
        "w1t": _bf16(W1.T),
        "b1d": np.ascontiguousarray(np.asarray(b1, np.float32).reshape(HC, 128).T),
        "weff": _bf16(weff),
        "beff": np.ascontiguousarray(beff.astype(np.float32).reshape(M_ALL, 1)),
        "linc": _bf16(linc),
        "diffc": _bf16(diffc),
    }

    x32 = np.asarray(input, np.float32)
    in_maps = []
    for c in range(N_CORES):
        m = dict(shared)
        xc = x32[c * B_SH:(c + 1) * B_SH, :]
        m["xb"] = _bf16(xc.T)
        m["ydy"] = _bf16(np.concatenate([xc[:, 7:16], xc[:, 22:31]], axis=1).T)
        in_maps.append(m)
    return in_maps


def kernel(input, W0, b0, W1, b1, Wl, bl):
    nc = _get_program()
    in_maps = _prepare_host_inputs(input, W0, b0, W1, b1, Wl, bl)
    results = run_bass_kernel_spmd(nc, in_maps, core_ids=list(range(N_CORES)))
    outs = []
    for c in range(N_CORES):
        o = results.results[c]["outT"]                     # (90, 2048)
        outs.append(o.reshape(DIM, NOUT, B_SH).transpose(2, 0, 1))
    return np.ascontiguousarray(np.concatenate(outs, axis=0), dtype=np.float32)
